# revision 11
# baseline (speedup 1.0000x reference)
"""HRAN-GNN Trainium2 kernel: 8-core SPMD, row-sharded attention + GNN.

v2 design (per core c, rows i = [512c, 512c+512), transposed layout
[j-part, i-free] everywhere):

Attention P = mask * exp(leaky(s_src_i + s_dst_j)) computed exactly by two
paths, split across engines per 4-chunk slab [128, 4, 512]:
  - ACT path: lr_k = Prelu(ssrc + bias sdst_j); E = Exp(lr slab) (slab-fused);
    P = E * adjT on DVE.  (Prelu + Exp + Tanh + Copy live in ONE act table
    set -> no table swaps; sigmoid later is 0.5 + 0.5*tanh(x/2).)
  - DVE path: exp(leaky(v)) = max(exp(v), exp(0.01 v)) and both factor as
    rank-1: E1 = exp(s_i)*exp(s_j) via tensor_scalar (bf16 4x mode),
    E2 likewise with the 0.01-scaled scores; M = max(E1,E2); P = M * adjT.
PE contracts P chunks against stationary [wh_r | ones] -> ht[65, 512] PSUM
(ones col gives softmax Z).  Z reciprocals via reciprocal_approx_fast.

GNN layers: sender-side projection (sup1T = Wg0^T h'^T, one matmul), cast,
4 XBAR DMA transposes -> [128, 4, 64] shard, AllGather, receivers use the
gathered tiles directly as aggregation stationaries against resident adjT
of `relation`.  deg comes from the ones col of agg1; dinv broadcast stays
in PSUM and is reused by layer 2.  Layer-2 gather payload is [128,4,32].
"""
import os
import sys
import types

sys.path.insert(0, "/opt/trn_rl_repo")
sys.path.insert(0, "/root/.axon_site")

from contextlib import ExitStack
import numpy as np
import ml_dtypes

import concourse.bass as bass
import concourse.tile as tile
from concourse import bacc, mybir
from concourse.bass_utils import run_bass_kernel_spmd

F32 = mybir.dt.float32
BF16 = mybir.dt.bfloat16
NPBF = ml_dtypes.bfloat16

N = 4096
IN_F = 256
H0, H1, H2 = 64, 64, 32
SLOPE = 0.01
N_CORES = 8
R = N // N_CORES          # 512 rows per core
NJC = N // 128            # 32 j-chunks
NSLAB = 8                 # 8 slabs of 4 chunks per relation
SLABC = 4                 # chunks per slab

# per-(relation, slab) path: True = ACT path, False = DVE path
PATH_ACT = [
    [True, True, False, True, False, True, False, True],
    [True, True, False, True, False, True, False, True],
    [True, False, False, True, False, True, False, True],
]
# ACT-path slabs whose mask-multiply runs on GpSimd (Pool) instead of DVE
POOL_MASK = {(0, 5), (1, 5), (2, 5), (0, 7)}

_model_cache = {}
DEBUG_DUMP = False


def _build_model():
    if "nc" in _model_cache:
        return _model_cache["nc"]
    nc = bacc.Bacc("TRN2", target_bir_lowering=False, debug=False,
                   num_devices=N_CORES)

    adjt = nc.dram_tensor("adjt", [3, NSLAB, 128, SLABC, R], BF16,
                          kind="ExternalInput").ap()
    whcat = nc.dram_tensor("whcat", [4, 128, 8, 200], BF16,
                           kind="ExternalInput").ap()
    ssrcb = nc.dram_tensor("ssrcb", [3, 128, R], F32, kind="ExternalInput").ap()
    sdst = nc.dram_tensor("sdst", [128, 96], F32, kind="ExternalInput").ap()
    expd = nc.dram_tensor("expd", [128, 96], F32, kind="ExternalInput").ap()
    expd001 = nc.dram_tensor("expd001", [128, 96], F32,
                             kind="ExternalInput").ap()
    wg0 = nc.dram_tensor("wg0", [H1, H1], BF16, kind="ExternalInput").ap()
    wg1 = nc.dram_tensor("wg1", [H1, H2], BF16, kind="ExternalInput").ap()
    wrt = nc.dram_tensor("wrt", [H1, H2], BF16, kind="ExternalInput").ap()
    bg0 = nc.dram_tensor("bg0", [H1, 1], F32, kind="ExternalInput").ap()
    bg1 = nc.dram_tensor("bg1", [H2, 1], F32, kind="ExternalInput").ap()
    brc = nc.dram_tensor("brc", [H2, 1], F32, kind="ExternalInput").ap()
    outT = nc.dram_tensor("outT", [H2, R], F32, kind="ExternalOutput").ap()
    if DEBUG_DUMP:
        dbgP0 = nc.dram_tensor("dbgP0", [128, R], F32, kind="ExternalOutput").ap()
        dbgP2 = nc.dram_tensor("dbgP2", [128, R], F32, kind="ExternalOutput").ap()
        dbgZ = nc.dram_tensor("dbgZ", [1, R], F32, kind="ExternalOutput").ap()
        dbgRZ = nc.dram_tensor("dbgRZ", [1, R], F32, kind="ExternalOutput").ap()
        dbgHP = nc.dram_tensor("dbgHP", [H0, R], F32, kind="ExternalOutput").ap()
        dbgS1L = nc.dram_tensor("dbgS1L", [128, SLABC, H1], F32, kind="ExternalOutput").ap()
        dbgS1A = nc.dram_tensor("dbgS1A", [128, H1 + 1], F32, kind="ExternalOutput").ap()
        dbgAGG = nc.dram_tensor("dbgAGG", [H1 + 1, R], F32, kind="ExternalOutput").ap()
        dbgH1P = nc.dram_tensor("dbgH1P", [H1, R], F32, kind="ExternalOutput").ap()
        dbgS2A = nc.dram_tensor("dbgS2A", [128, H2], F32, kind="ExternalOutput").ap()
        dbgAGG2 = nc.dram_tensor("dbgAGG2", [H2, R], F32, kind="ExternalOutput").ap()
        dbgRES = nc.dram_tensor("dbgRES", [H2, R], F32, kind="ExternalOutput").ap()
        dbgT2 = nc.dram_tensor("dbgT2", [H2, R], F32, kind="ExternalOutput").ap()

    cc1_in = nc.dram_tensor("cc1_in", [128, SLABC, H1], BF16).ap()
    cc1_out = nc.dram_tensor("cc1_out", [N_CORES, 128, SLABC, H1], BF16,
                             addr_space="Shared").ap()
    ccw_in = nc.dram_tensor("ccw_in", [1, 16], BF16).ap()
    ccw_out = nc.dram_tensor("ccw_out", [N_CORES, 1, 16], BF16,
                             addr_space="Shared").ap()
    cc2_in = nc.dram_tensor("cc2_in", [128, SLABC, H2], BF16).ap()
    cc2_out = nc.dram_tensor("cc2_out", [N_CORES, 128, SLABC, H2], BF16,
                             addr_space="Shared").ap()
    groups = [list(range(N_CORES))]

    PRELU = mybir.ActivationFunctionType.Prelu
    EXP = mybir.ActivationFunctionType.Exp
    TANH = mybir.ActivationFunctionType.Tanh
    CPY = mybir.ActivationFunctionType.Copy
    MULT = mybir.AluOpType.mult
    ADD = mybir.AluOpType.add

    with tile.TileContext(nc) as tc, ExitStack() as ctx:
        resid = ctx.enter_context(tc.tile_pool(name="resid", bufs=1))
        small = ctx.enter_context(tc.tile_pool(name="small", bufs=1))
        dbg = 1 if DEBUG_DUMP else 0
        stream = ctx.enter_context(tc.tile_pool(name="stream", bufs=4 - 2 * dbg))
        lrp = ctx.enter_context(tc.tile_pool(name="lrp", bufs=3 - dbg))
        ep = ctx.enter_context(tc.tile_pool(name="ep", bufs=3 - dbg))
        e12p = ctx.enter_context(tc.tile_pool(name="e12p", bufs=2))
        mp = ctx.enter_context(tc.tile_pool(name="mp", bufs=2))
        pp = ctx.enter_context(tc.tile_pool(name="pp", bufs=3 - dbg))
        seq = ctx.enter_context(tc.tile_pool(name="seq", bufs=1))

        # warm up the collective rings early (first CC op pays ~11us launch
        # + slow transfer; do it on garbage while attention runs)
        nc.gpsimd.collective_compute("AllGather", mybir.AluOpType.bypass,
                                     replica_groups=groups,
                                     ins=[ccw_in[:]], outs=[ccw_out[:]])

        # ---- resident loads -------------------------------------------------
        adjres = resid.tile([128, NSLAB, SLABC, R], BF16)     # relation's adjT
        nc.sync.dma_start(adjres[:, 0], adjt[0, 0])
        wh_sb = resid.tile([128, 32, 200], BF16)
        for s4 in range(4):
            nc.sync.dma_start(wh_sb[:, s4 * 8:(s4 + 1) * 8, :], whcat[s4])
        ssrc_sb = [resid.tile([128, R], F32, tag=f"ssrc{ri}", name=f"ssrc{ri}")
                   for ri in range(3)]
        for ri in range(3):
            nc.sync.dma_start(ssrc_sb[ri][:], ssrcb[ri])
        sdst_sb = resid.tile([128, 96], F32)
        nc.sync.dma_start(sdst_sb[:], sdst[:])
        expd_sb = resid.tile([128, 96], F32)
        nc.sync.dma_start(expd_sb[:], expd[:])
        expd001_sb = resid.tile([128, 96], F32)
        nc.sync.dma_start(expd001_sb[:], expd001[:])
        for s in range(1, NSLAB):
            nc.sync.dma_start(adjres[:, s], adjt[0, s])

        wg0_sb = small.tile([H1, H1], BF16, tag="wg0")
        nc.sync.dma_start(wg0_sb[:], wg0[:])
        wg1_sb = small.tile([H1, H2], BF16, tag="wg1")
        nc.sync.dma_start(wg1_sb[:], wg1[:])
        wrt_sb = small.tile([H1, H2], BF16, tag="wrt")
        nc.sync.dma_start(wrt_sb[:], wrt[:])
        bg0_sb = small.tile([H1, 1], F32, tag="bg0")
        nc.sync.dma_start(bg0_sb[:], bg0[:])
        bg1_sb = small.tile([H2, 1], F32, tag="bg1")
        nc.sync.dma_start(bg1_sb[:], bg1[:])
        brc_sb = small.tile([H2, 1], F32, tag="brc")
        nc.sync.dma_start(brc_sb[:], brc[:])
        onec = small.tile([1, H1], BF16, tag="onec")
        nc.vector.memset(onec[:], 1.0)

        # rank-1 factors for the DVE path, built on device:
        #   A_b = exp(s_src) broadcast, a_b = exp(0.01 s_src) broadcast
        A_b = [resid.tile([128, R], BF16, tag=f"Ab{ri}", name=f"Ab{ri}")
               for ri in range(3)]
        a_b = [resid.tile([128, R], BF16, tag=f"ab{ri}", name=f"ab{ri}")
               for ri in range(3)]
        for ri in range(3):
            nc.scalar.activation(A_b[ri][:], ssrc_sb[ri][:], EXP)
            nc.scalar.activation(a_b[ri][:], ssrc_sb[ri][:], EXP, scale=0.01)

        # gathered aggregation stationaries (ones col for deg in layer 1)
        sup1_all = resid.tile([128, NJC, H1 + 1], BF16)
        nc.vector.memset(sup1_all[:, :, H1:H1 + 1], 1.0)
        sup2_all = resid.tile([128, NJC, H2], BF16)

        # ---- phase A: masked-softmax attention, all 3 relations -------------
        ps_ctx = ExitStack()
        psA = ps_ctx.enter_context(tc.tile_pool(name="psA", bufs=1,
                                                space="PSUM"))
        psR = ps_ctx.enter_context(tc.tile_pool(name="psR", bufs=2,
                                                space="PSUM"))
        ht = [psA.tile([H0 + 1, R], F32, tag=f"ht{ri}", name=f"ht{ri}")
              for ri in range(3)]
        for ri in range(3):
            for s in range(NSLAB):
                if ri == 0:
                    at = adjres[:, s]
                else:
                    att = stream.tile([128, SLABC, R], BF16, tag="adj_stream",
                                      name=f"adj_{ri}_{s}")
                    nc.sync.dma_start(att[:], adjt[ri, s])
                    at = att[:]
                p = pp.tile([128, SLABC, R], BF16, tag="p", name=f"p_{ri}_{s}")
                if PATH_ACT[ri][s]:
                    lr = lrp.tile([128, SLABC, R], F32, tag="lr",
                                  name=f"lr_{ri}_{s}")
                    for k in range(SLABC):
                        col = ri * 32 + s * SLABC + k
                        nc.scalar.activation(
                            lr[:, k, :], ssrc_sb[ri][:], PRELU,
                            bias=sdst_sb[:, col:col + 1], scale=1.0,
                            alpha=SLOPE)
                    ex = ep.tile([128, SLABC, R], BF16, tag="ex",
                                 name=f"ex_{ri}_{s}")
                    nc.scalar.activation(ex[:], lr[:], EXP)
                    if (ri, s) in POOL_MASK:
                        nc.gpsimd.tensor_mul(p[:], ex[:], at)
                    else:
                        nc.vector.tensor_mul(p[:], ex[:], at)
                else:
                    e1 = e12p.tile([128, SLABC, R], BF16, tag="e1",
                                   name=f"e1_{ri}_{s}")
                    e2 = e12p.tile([128, SLABC, R], BF16, tag="e2",
                                   name=f"e2_{ri}_{s}")
                    for k in range(SLABC):
                        col = ri * 32 + s * SLABC + k
                        nc.vector.tensor_scalar(
                            e1[:, k, :], A_b[ri][:],
                            expd_sb[:, col:col + 1], None, MULT)
                        nc.vector.tensor_scalar(
                            e2[:, k, :], a_b[ri][:],
                            expd001_sb[:, col:col + 1], None, MULT)
                    m = mp.tile([128, SLABC, R], BF16, tag="m",
                                name=f"m_{ri}_{s}")
                    nc.vector.tensor_max(m[:], e1[:], e2[:])
                    nc.vector.tensor_mul(p[:], m[:], at)
                if DEBUG_DUMP and ri == 0 and s in (0, 2):
                    dP = seq.tile([128, R], F32, tag=f"dP{s}")
                    nc.vector.tensor_scalar(dP[:], p[:, 0, :], 1.0, None, MULT)
                    nc.sync.dma_start(dbgP0 if s == 0 else dbgP2, dP[:])
                for k in range(SLABC):
                    jc = s * SLABC + k
                    nc.tensor.matmul(ht[ri][:],
                                     wh_sb[:, jc, ri * 65:ri * 65 + 65],
                                     p[:, k, :],
                                     start=(jc == 0), stop=(jc == NJC - 1))

        # combine: h' = sigmoid(mean of normalized heads) = 0.5+0.5tanh(x/2)
        msum = None
        for ri in range(3):
            zin = seq.tile([1, R], F32, tag=f"zin{ri}")
            nc.scalar.activation(zin[:], ht[ri][H0:H0 + 1, :], CPY)
            rz = seq.tile([1, R], F32, tag=f"rz{ri}")
            nc.vector.reciprocal_approx_fast(rz[:], zin[:])
            if DEBUG_DUMP and ri == 0:
                dZ = seq.tile([1, R], F32, tag="dZ")
                nc.scalar.activation(dZ[:], ht[ri][H0:H0 + 1, :], CPY)
                nc.sync.dma_start(dbgZ, dZ[:])
                nc.sync.dma_start(dbgRZ, rz[:])
            rz16 = seq.tile([1, R], BF16, tag=f"rz16{ri}")
            nc.vector.tensor_scalar(rz16[:], rz[:], 1.0 / 3.0, None, MULT)
            rzb_ps = psR.tile([H0, R], F32, tag="rzb")
            nc.tensor.matmul(rzb_ps[:], onec[:, 0:H0], rz16[:],
                             start=True, stop=True)
            rzb = seq.tile([H0, R], F32, tag=f"rzb_sb{ri}")
            nc.scalar.activation(rzb[:], rzb_ps[:], CPY)
            m = seq.tile([H0, R], F32, tag=f"m{ri}")
            nc.vector.tensor_mul(m[:], rzb[:], ht[ri][0:H0, :])
            if msum is None:
                msum = m
            else:
                m2 = seq.tile([H0, R], F32, tag=f"msum{ri}")
                nc.vector.tensor_add(m2[:], msum[:], m[:])
                msum = m2
        th = seq.tile([H0, R], BF16, tag="th")
        nc.scalar.activation(th[:], msum[:], TANH, scale=0.5)
        hpT = seq.tile([H0, R], BF16, tag="hpT")
        nc.vector.tensor_scalar(hpT[:], th[:], 0.5, 0.5, MULT, ADD)
        if DEBUG_DUMP:
            dHP = seq.tile([H0, R], F32, tag="dHP")
            nc.vector.tensor_scalar(dHP[:], hpT[:], 1.0, None, MULT)
            nc.sync.dma_start(dbgHP, dHP[:])
        ps_ctx.close()

        # ---- layer 1: project locally, transpose, AllGather, aggregate ------
        psB = ctx.enter_context(tc.tile_pool(name="psB", bufs=1, space="PSUM"))
        sup1p = psB.tile([H1, R], F32, tag="sup1p")
        nc.tensor.matmul(sup1p[:], wg0_sb[:], hpT[:], start=True, stop=True)
        sup1T = seq.tile([H1, R], BF16, tag="sup1T")
        nc.scalar.activation(sup1T[:], sup1p[:], CPY)
        sup1L = seq.tile([128, SLABC, H1], BF16, tag="sup1L")
        for q in range(SLABC):
            eng = nc.sync if q % 2 == 0 else nc.scalar
            eng.dma_start_transpose(sup1L[:, q, :],
                                    sup1T[:, q * 128:(q + 1) * 128])
        if DEBUG_DUMP:
            dS1L = seq.tile([128, SLABC, H1], F32, tag="dS1L")
            nc.vector.tensor_scalar(dS1L[:], sup1L[:], 1.0, None, MULT)
            nc.sync.dma_start(dbgS1L, dS1L[:])
        nc.sync.dma_start(cc1_in[:], sup1L[:])
        nc.gpsimd.collective_compute("AllGather", mybir.AluOpType.bypass,
                                     replica_groups=groups,
                                     ins=[cc1_in[:]], outs=[cc1_out[:]])
        for c in range(N_CORES):
            nc.sync.dma_start(sup1_all[:, c * SLABC:(c + 1) * SLABC, 0:H1],
                              cc1_out[c])

        if DEBUG_DUMP:
            dS1A = seq.tile([128, H1 + 1], F32, tag="dS1A")
            nc.vector.tensor_scalar(dS1A[:], sup1_all[:, 0, :], 1.0, None, MULT)
            nc.sync.dma_start(dbgS1A, dS1A[:])
        agg1 = psB.tile([H1 + 1, R], F32, tag="agg1")
        for jc in range(NJC):
            nc.tensor.matmul(agg1[:], sup1_all[:, jc, :],
                             adjres[:, jc // SLABC, jc % SLABC, :],
                             start=(jc == 0), stop=(jc == NJC - 1))
        degin = seq.tile([1, R], F32, tag="degin")
        nc.scalar.activation(degin[:], agg1[H1:H1 + 1, :], CPY)
        dinv = seq.tile([1, R], F32, tag="dinv")
        nc.vector.reciprocal_approx_fast(dinv[:], degin[:])
        dinv16 = seq.tile([1, R], BF16, tag="dinv16")
        nc.vector.tensor_scalar(dinv16[:], dinv[:], 1.0, None, MULT)
        dinvb_ps = psB.tile([H1, R], F32, tag="dinvb")
        nc.tensor.matmul(dinvb_ps[:], onec[:, 0:H1], dinv16[:],
                         start=True, stop=True)
        dinvb = seq.tile([H1, R], F32, tag="dinvb_sb")
        nc.scalar.activation(dinvb[:], dinvb_ps[:], CPY)
        m1 = seq.tile([H1, R], F32, tag="l1m")
        nc.vector.tensor_mul(m1[:], dinvb[:], agg1[0:H1, :])
        h1pT = seq.tile([H1, R], BF16, tag="h1pT")
        nc.scalar.activation(h1pT[:], m1[:], PRELU, bias=bg0_sb[:], scale=1.0,
                             alpha=SLOPE)
        if DEBUG_DUMP:
            dAGG = seq.tile([H1 + 1, R], F32, tag="dAGG")
            nc.scalar.activation(dAGG[:], agg1[:], CPY)
            nc.sync.dma_start(dbgAGG, dAGG[:])
            dH1P = seq.tile([H1, R], F32, tag="dH1P")
            nc.vector.tensor_scalar(dH1P[:], h1pT[:], 1.0, None, MULT)
            nc.sync.dma_start(dbgH1P, dH1P[:])

        # ---- layer 2 + residual --------------------------------------------
        sup2p = psB.tile([H2, R], F32, tag="sup2p")
        nc.tensor.matmul(sup2p[:], wg1_sb[:], h1pT[:], start=True, stop=True)
        sup2T = seq.tile([H2, R], BF16, tag="sup2T")
        nc.scalar.activation(sup2T[:], sup2p[:], CPY)
        sup2L = seq.tile([128, SLABC, H2], BF16, tag="sup2L")
        for q in range(SLABC):
            eng = nc.sync if q % 2 == 0 else nc.scalar
            eng.dma_start_transpose(sup2L[:, q, :],
                                    sup2T[:, q * 128:(q + 1) * 128])
        nc.sync.dma_start(cc2_in[:], sup2L[:])
        nc.gpsimd.collective_compute("AllGather", mybir.AluOpType.bypass,
                                     replica_groups=groups,
                                     ins=[cc2_in[:]], outs=[cc2_out[:]])
        for c in range(N_CORES):
            nc.sync.dma_start(sup2_all[:, c * SLABC:(c + 1) * SLABC, :],
                              cc2_out[c])

        if DEBUG_DUMP:
            dS2A = seq.tile([128, H2], F32, tag="dS2A")
            nc.vector.tensor_scalar(dS2A[:], sup2_all[:, 0, :], 1.0, None, MULT)
            nc.sync.dma_start(dbgS2A, dS2A[:])
        agg2 = psB.tile([H2, R], F32, tag="agg2")
        for jc in range(NJC):
            nc.tensor.matmul(agg2[:], sup2_all[:, jc, :],
                             adjres[:, jc // SLABC, jc % SLABC, :],
                             start=(jc == 0), stop=(jc == NJC - 1))
        resT = psB.tile([H2, R], F32, tag="resT")
        nc.tensor.matmul(resT[:], wrt_sb[:], h1pT[:], start=True, stop=True)

        m2t = seq.tile([H2, R], F32, tag="l2m")
        nc.vector.tensor_mul(m2t[:], dinvb[0:H2, :], agg2[:])
        t2 = seq.tile([H2, R], F32, tag="t2")
        nc.scalar.activation(t2[:], m2t[:], PRELU, bias=bg1_sb[:], scale=1.0,
                             alpha=SLOPE)
        if DEBUG_DUMP:
            dAGG2 = seq.tile([H2, R], F32, tag="dAGG2")
            nc.scalar.activation(dAGG2[:], agg2[:], CPY)
            nc.sync.dma_start(dbgAGG2, dAGG2[:])
            dRES = seq.tile([H2, R], F32, tag="dRES")
            nc.scalar.activation(dRES[:], resT[:], CPY)
            nc.sync.dma_start(dbgRES, dRES[:])
            nc.sync.dma_start(dbgT2, t2[:])
        fin = seq.tile([H2, R], F32, tag="fin")
        nc.vector.tensor_add(fin[:], t2[:], resT[:])
        fin2 = seq.tile([H2, R], F32, tag="fin2")
        nc.vector.tensor_scalar(fin2[:], fin[:], brc_sb[:], None, ADD)
        nc.sync.dma_start(outT[:], fin2[:])

    nc.compile()
    _model_cache["nc"] = nc
    return nc


def kernel(x, adj, W1, a1, W2, a2, W3, a3, Wg0, bg0, Wg1, bg1, Wr, br,
           relation):
    x = np.asarray(x, dtype=np.float32)
    adj = np.asarray(adj, dtype=np.float32)
    rel = int(np.asarray(relation))
    rel_list = [rel] + [r for r in range(3) if r != rel]
    Ws = [np.asarray(W, np.float32) for W in (W1, W2, W3)]
    As = [np.asarray(a, np.float32) for a in (a1, a2, a3)]

    # host prep: projections and score vectors (small)
    wh = [x @ Ws[r] for r in range(3)]                      # [N, 64] each
    s_src = [wh[r] @ As[r][:H0, 0] for r in range(3)]       # [N]
    s_dst = [wh[r] @ As[r][H0:, 0] for r in range(3)]       # [N]

    whcat = np.zeros((N, 200), np.float32)
    for ri, r in enumerate(rel_list):
        whcat[:, ri * 65:ri * 65 + 64] = wh[r]
        whcat[:, ri * 65 + 64] = 1.0
    # [4, 128, 8, 200]: row j = s4*1024 + k*128 + p
    whcat_sw = np.ascontiguousarray(
        whcat.astype(NPBF).reshape(4, 8, 128, 200).transpose(0, 2, 1, 3))

    # per-(relation, chunk) per-partition scalars [128, 96]
    sdst_c = np.concatenate(
        [s_dst[r].reshape(NJC, 128).T for r in rel_list], axis=1)
    sdst_c = np.ascontiguousarray(sdst_c.astype(np.float32))
    expd_c = np.exp(sdst_c).astype(np.float32)
    expd001_c = np.exp(0.01 * sdst_c).astype(np.float32)

    adj_bf = adj.astype(NPBF)
    in_maps = []
    for c in range(N_CORES):
        rows = slice(c * R, (c + 1) * R)
        # adjT[j, i] with j = s*512 + k*128 + p -> [3, 8, 128, 4, 512]
        adjt_c = adj_bf[rel_list][:, rows, :].transpose(0, 2, 1)  # [3,4096,512]
        adjt_c = np.ascontiguousarray(
            adjt_c.reshape(3, NSLAB, SLABC, 128, R).transpose(0, 1, 3, 2, 4))
        ssrcb_c = np.ascontiguousarray(np.broadcast_to(
            np.stack([s_src[r][rows] for r in rel_list])[:, None, :],
            (3, 128, R))).astype(np.float32)
        in_maps.append({
            "adjt": adjt_c,
            "whcat": whcat_sw,
            "ssrcb": ssrcb_c,
            "sdst": sdst_c,
            "expd": expd_c,
            "expd001": expd001_c,
            "wg0": np.asarray(Wg0, np.float32).astype(NPBF),
            "wg1": np.asarray(Wg1, np.float32).astype(NPBF),
            "wrt": np.ascontiguousarray(
                np.asarray(Wr, np.float32).T).astype(NPBF),
            "bg0": np.asarray(bg0, np.float32).reshape(H1, 1),
            "bg1": np.asarray(bg1, np.float32).reshape(H2, 1),
            "brc": np.asarray(br, np.float32).reshape(H2, 1),
        })

    nc = _build_model()
    kw = {}
    if os.environ.get("HRAN_TRACE"):
        _install_hook()
        kw = dict(trace=True, tmpdir=os.environ.get("HRAN_TRACE_DIR") or None)
    res = run_bass_kernel_spmd(nc, in_maps, core_ids=list(range(N_CORES)), **kw)
    if os.environ.get("HRAN_TRACE"):
        print(f"HW exec time: {res.exec_time_ns} ns")
    out = np.concatenate(
        [np.asarray(res.results[c]["outT"], np.float32).T
         for c in range(N_CORES)],
        axis=0)
    return out


def _install_hook():
    import antenv
    if "antenv.axon_hooks" in sys.modules:
        return
    from trn_agent_boot.trn_boot import _ntff_profile_via_ctypes
    hook = _ntff_profile_via_ctypes("/opt/axon/libaxon_pjrt.so")
    mod = types.ModuleType("antenv.axon_hooks")
    mod.get_axon_ntff_profile_hook = lambda: hook
    mod.set_axon_ntff_profile_hook = lambda h: None
    sys.modules["antenv.axon_hooks"] = mod
    antenv.axon_hooks = mod


# revision 12
# speedup vs baseline: 1.1969x; 1.1969x over previous
"""HRAN-GNN Trainium2 kernel: 8-core SPMD, row-sharded attention + GNN.

v2 design (per core c, rows i = [512c, 512c+512), transposed layout
[j-part, i-free] everywhere):

Attention P = mask * exp(leaky(s_src_i + s_dst_j)) computed exactly by two
paths, split across engines per 4-chunk slab [128, 4, 512]:
  - ACT path: lr_k = Prelu(ssrc + bias sdst_j); E = Exp(lr slab) (slab-fused);
    P = E * adjT on DVE.  (Prelu + Exp + Tanh + Copy live in ONE act table
    set -> no table swaps; sigmoid later is 0.5 + 0.5*tanh(x/2).)
  - DVE path: exp(leaky(v)) = max(exp(v), exp(0.01 v)) and both factor as
    rank-1: E1 = exp(s_i)*exp(s_j) via tensor_scalar (bf16 4x mode),
    E2 likewise with the 0.01-scaled scores; M = max(E1,E2); P = M * adjT.
PE contracts P chunks against stationary [wh_r | ones] -> ht[65, 512] PSUM
(ones col gives softmax Z).  Z reciprocals via reciprocal_approx_fast.

GNN layers: sender-side projection (sup1T = Wg0^T h'^T, one matmul), cast,
4 XBAR DMA transposes -> [128, 4, 64] shard, AllGather, receivers use the
gathered tiles directly as aggregation stationaries against resident adjT
of `relation`.  deg comes from the ones col of agg1; dinv broadcast stays
in PSUM and is reused by layer 2.  Layer-2 gather payload is [128,4,32].
"""
import os
import sys
import types

sys.path.insert(0, "/opt/trn_rl_repo")
sys.path.insert(0, "/root/.axon_site")

from contextlib import ExitStack
import numpy as np
import ml_dtypes

import concourse.bass as bass
import concourse.tile as tile
from concourse import bacc, mybir
from concourse.bass_utils import run_bass_kernel_spmd

F32 = mybir.dt.float32
BF16 = mybir.dt.bfloat16
NPBF = ml_dtypes.bfloat16

N = 4096
IN_F = 256
H0, H1, H2 = 64, 64, 32
SLOPE = 0.01
N_CORES = 8
R = N // N_CORES          # 512 rows per core
NJC = N // 128            # 32 j-chunks
NSLAB = 8                 # 8 slabs of 4 chunks per relation
SLABC = 4                 # chunks per slab

# per-(relation, slab) path: True = ACT path, False = DVE path
PATH_ACT = [
    [True, True, False, True, False, True, False, True],
    [True, True, False, True, False, True, False, True],
    [True, False, False, True, False, True, False, True],
]
# ACT-path slabs whose mask-multiply runs on GpSimd (Pool) instead of DVE
POOL_MASK = set()

_model_cache = {}
DEBUG_DUMP = False


def _build_model():
    if "nc" in _model_cache:
        return _model_cache["nc"]
    nc = bacc.Bacc("TRN2", target_bir_lowering=False, debug=False,
                   num_devices=N_CORES)

    adjt = nc.dram_tensor("adjt", [3, NSLAB, 128, SLABC, R], BF16,
                          kind="ExternalInput").ap()
    whcat = nc.dram_tensor("whcat", [4, 128, 8, 200], BF16,
                           kind="ExternalInput").ap()
    ssrcb = nc.dram_tensor("ssrcb", [3, 128, R], F32, kind="ExternalInput").ap()
    sdst = nc.dram_tensor("sdst", [128, 96], F32, kind="ExternalInput").ap()
    expd = nc.dram_tensor("expd", [128, 96], F32, kind="ExternalInput").ap()
    expd001 = nc.dram_tensor("expd001", [128, 96], F32,
                             kind="ExternalInput").ap()
    wg0 = nc.dram_tensor("wg0", [H1, H1], BF16, kind="ExternalInput").ap()
    wg1 = nc.dram_tensor("wg1", [H1, H2], BF16, kind="ExternalInput").ap()
    wrt = nc.dram_tensor("wrt", [H1, H2], BF16, kind="ExternalInput").ap()
    bg0 = nc.dram_tensor("bg0", [H1, 1], F32, kind="ExternalInput").ap()
    bg1 = nc.dram_tensor("bg1", [H2, 1], F32, kind="ExternalInput").ap()
    brc = nc.dram_tensor("brc", [H2, 1], F32, kind="ExternalInput").ap()
    outT = nc.dram_tensor("outT", [H2, R], F32, kind="ExternalOutput").ap()
    if DEBUG_DUMP:
        dbgP0 = nc.dram_tensor("dbgP0", [128, R], F32, kind="ExternalOutput").ap()
        dbgP2 = nc.dram_tensor("dbgP2", [128, R], F32, kind="ExternalOutput").ap()
        dbgZ = nc.dram_tensor("dbgZ", [1, R], F32, kind="ExternalOutput").ap()
        dbgRZ = nc.dram_tensor("dbgRZ", [1, R], F32, kind="ExternalOutput").ap()
        dbgHP = nc.dram_tensor("dbgHP", [H0, R], F32, kind="ExternalOutput").ap()
        dbgS1L = nc.dram_tensor("dbgS1L", [128, SLABC, H1], F32, kind="ExternalOutput").ap()
        dbgS1A = nc.dram_tensor("dbgS1A", [128, H1 + 1], F32, kind="ExternalOutput").ap()
        dbgAGG = nc.dram_tensor("dbgAGG", [H1 + 1, R], F32, kind="ExternalOutput").ap()
        dbgH1P = nc.dram_tensor("dbgH1P", [H1, R], F32, kind="ExternalOutput").ap()
        dbgS2A = nc.dram_tensor("dbgS2A", [128, H2], F32, kind="ExternalOutput").ap()
        dbgAGG2 = nc.dram_tensor("dbgAGG2", [H2, R], F32, kind="ExternalOutput").ap()
        dbgRES = nc.dram_tensor("dbgRES", [H2, R], F32, kind="ExternalOutput").ap()
        dbgT2 = nc.dram_tensor("dbgT2", [H2, R], F32, kind="ExternalOutput").ap()

    cc1_in = nc.dram_tensor("cc1_in", [128, SLABC, H1], BF16).ap()
    cc1_out = nc.dram_tensor("cc1_out", [N_CORES, 128, SLABC, H1], BF16,
                             addr_space="Shared").ap()
    ccw_in = nc.dram_tensor("ccw_in", [1, 16], BF16).ap()
    ccw_out = nc.dram_tensor("ccw_out", [N_CORES, 1, 16], BF16,
                             addr_space="Shared").ap()
    cc2_in = nc.dram_tensor("cc2_in", [128, SLABC, H2], BF16).ap()
    cc2_out = nc.dram_tensor("cc2_out", [N_CORES, 128, SLABC, H2], BF16,
                             addr_space="Shared").ap()
    groups = [list(range(N_CORES))]

    PRELU = mybir.ActivationFunctionType.Prelu
    EXP = mybir.ActivationFunctionType.Exp
    TANH = mybir.ActivationFunctionType.Tanh
    CPY = mybir.ActivationFunctionType.Copy
    MULT = mybir.AluOpType.mult
    ADD = mybir.AluOpType.add

    with tile.TileContext(nc) as tc, ExitStack() as ctx:
        resid = ctx.enter_context(tc.tile_pool(name="resid", bufs=1))
        small = ctx.enter_context(tc.tile_pool(name="small", bufs=1))
        dbg = 1 if DEBUG_DUMP else 0
        stream = ctx.enter_context(tc.tile_pool(name="stream", bufs=4 - 2 * dbg))
        lrp = ctx.enter_context(tc.tile_pool(name="lrp", bufs=3 - dbg))
        ep = ctx.enter_context(tc.tile_pool(name="ep", bufs=3 - dbg))
        e12p = ctx.enter_context(tc.tile_pool(name="e12p", bufs=2))
        mp = ctx.enter_context(tc.tile_pool(name="mp", bufs=2))
        pp = ctx.enter_context(tc.tile_pool(name="pp", bufs=3 - dbg))
        seq = ctx.enter_context(tc.tile_pool(name="seq", bufs=1))

        # warm up the collective rings early (first CC op pays ~11us launch
        # + slow transfer; do it on garbage while attention runs)
        nc.gpsimd.collective_compute("AllGather", mybir.AluOpType.bypass,
                                     replica_groups=groups,
                                     ins=[ccw_in[:]], outs=[ccw_out[:]])

        # ---- resident loads -------------------------------------------------
        adjres = resid.tile([128, NSLAB, SLABC, R], BF16)     # relation's adjT
        nc.sync.dma_start(adjres[:, 0], adjt[0, 0])
        wh_sb = resid.tile([128, 32, 200], BF16)
        for s4 in range(4):
            nc.sync.dma_start(wh_sb[:, s4 * 8:(s4 + 1) * 8, :], whcat[s4])
        ssrc_sb = [resid.tile([128, R], F32, tag=f"ssrc{ri}", name=f"ssrc{ri}")
                   for ri in range(3)]
        for ri in range(3):
            nc.sync.dma_start(ssrc_sb[ri][:], ssrcb[ri])
        sdst_sb = resid.tile([128, 96], F32)
        nc.sync.dma_start(sdst_sb[:], sdst[:])
        expd_sb = resid.tile([128, 96], F32)
        nc.sync.dma_start(expd_sb[:], expd[:])
        expd001_sb = resid.tile([128, 96], F32)
        nc.sync.dma_start(expd001_sb[:], expd001[:])
        for s in range(1, NSLAB):
            nc.sync.dma_start(adjres[:, s], adjt[0, s])

        wg0_sb = small.tile([H1, H1], BF16, tag="wg0")
        nc.sync.dma_start(wg0_sb[:], wg0[:])
        wg1_sb = small.tile([H1, H2], BF16, tag="wg1")
        nc.sync.dma_start(wg1_sb[:], wg1[:])
        wrt_sb = small.tile([H1, H2], BF16, tag="wrt")
        nc.sync.dma_start(wrt_sb[:], wrt[:])
        bg0_sb = small.tile([H1, 1], F32, tag="bg0")
        nc.sync.dma_start(bg0_sb[:], bg0[:])
        bg1_sb = small.tile([H2, 1], F32, tag="bg1")
        nc.sync.dma_start(bg1_sb[:], bg1[:])
        brc_sb = small.tile([H2, 1], F32, tag="brc")
        nc.sync.dma_start(brc_sb[:], brc[:])
        onec = small.tile([1, H1], BF16, tag="onec")
        nc.vector.memset(onec[:], 1.0)

        # rank-1 factors for the DVE path, built on device:
        #   A_b = exp(s_src) broadcast, a_b = exp(0.01 s_src) broadcast
        A_b = [resid.tile([128, R], BF16, tag=f"Ab{ri}", name=f"Ab{ri}")
               for ri in range(3)]
        a_b = [resid.tile([128, R], BF16, tag=f"ab{ri}", name=f"ab{ri}")
               for ri in range(3)]
        for ri in range(3):
            nc.scalar.activation(A_b[ri][:], ssrc_sb[ri][:], EXP)
            nc.scalar.activation(a_b[ri][:], ssrc_sb[ri][:], EXP, scale=0.01)

        # gathered aggregation stationaries (ones col for deg in layer 1)
        sup1_all = resid.tile([128, NJC, H1 + 1], BF16)
        nc.vector.memset(sup1_all[:, :, H1:H1 + 1], 1.0)
        sup2_all = resid.tile([128, NJC, H2], BF16)

        # ---- phase A: masked-softmax attention, all 3 relations -------------
        ps_ctx = ExitStack()
        psA = ps_ctx.enter_context(tc.tile_pool(name="psA", bufs=1,
                                                space="PSUM"))
        psR = ps_ctx.enter_context(tc.tile_pool(name="psR", bufs=2,
                                                space="PSUM"))
        ht = [psA.tile([H0 + 1, R], F32, tag=f"ht{ri}", name=f"ht{ri}")
              for ri in range(3)]
        for ri in range(3):
            for s in range(NSLAB):
                if ri == 0:
                    at = adjres[:, s]
                else:
                    att = stream.tile([128, SLABC, R], BF16, tag="adj_stream",
                                      name=f"adj_{ri}_{s}")
                    nc.sync.dma_start(att[:], adjt[ri, s])
                    at = att[:]
                p = pp.tile([128, SLABC, R], BF16, tag="p", name=f"p_{ri}_{s}")
                if PATH_ACT[ri][s]:
                    lr = lrp.tile([128, SLABC, R], F32, tag="lr",
                                  name=f"lr_{ri}_{s}")
                    for k in range(SLABC):
                        col = ri * 32 + s * SLABC + k
                        nc.scalar.activation(
                            lr[:, k, :], ssrc_sb[ri][:], PRELU,
                            bias=sdst_sb[:, col:col + 1], scale=1.0,
                            alpha=SLOPE)
                    ex = ep.tile([128, SLABC, R], BF16, tag="ex",
                                 name=f"ex_{ri}_{s}")
                    nc.scalar.activation(ex[:], lr[:], EXP)
                    if (ri, s) in POOL_MASK:
                        nc.gpsimd.tensor_mul(p[:], ex[:], at)
                    else:
                        nc.vector.tensor_mul(p[:], ex[:], at)
                else:
                    e1 = e12p.tile([128, SLABC, R], BF16, tag="e1",
                                   name=f"e1_{ri}_{s}")
                    e2 = e12p.tile([128, SLABC, R], BF16, tag="e2",
                                   name=f"e2_{ri}_{s}")
                    for k in range(SLABC):
                        col = ri * 32 + s * SLABC + k
                        nc.vector.tensor_scalar(
                            e1[:, k, :], A_b[ri][:],
                            expd_sb[:, col:col + 1], None, MULT)
                        nc.vector.tensor_scalar(
                            e2[:, k, :], a_b[ri][:],
                            expd001_sb[:, col:col + 1], None, MULT)
                    m = mp.tile([128, SLABC, R], BF16, tag="m",
                                name=f"m_{ri}_{s}")
                    nc.vector.tensor_max(m[:], e1[:], e2[:])
                    nc.vector.tensor_mul(p[:], m[:], at)
                if DEBUG_DUMP and ri == 0 and s in (0, 2):
                    dP = seq.tile([128, R], F32, tag=f"dP{s}")
                    nc.vector.tensor_scalar(dP[:], p[:, 0, :], 1.0, None, MULT)
                    nc.sync.dma_start(dbgP0 if s == 0 else dbgP2, dP[:])
                for k in range(SLABC):
                    jc = s * SLABC + k
                    nc.tensor.matmul(ht[ri][:],
                                     wh_sb[:, jc, ri * 65:ri * 65 + 65],
                                     p[:, k, :],
                                     start=(jc == 0), stop=(jc == NJC - 1))

        # combine: h' = sigmoid(mean of normalized heads) = 0.5+0.5tanh(x/2)
        msum = None
        for ri in range(3):
            zin = seq.tile([1, R], F32, tag=f"zin{ri}")
            nc.scalar.activation(zin[:], ht[ri][H0:H0 + 1, :], CPY)
            rz = seq.tile([1, R], F32, tag=f"rz{ri}")
            nc.vector.reciprocal_approx_fast(rz[:], zin[:])
            if DEBUG_DUMP and ri == 0:
                dZ = seq.tile([1, R], F32, tag="dZ")
                nc.scalar.activation(dZ[:], ht[ri][H0:H0 + 1, :], CPY)
                nc.sync.dma_start(dbgZ, dZ[:])
                nc.sync.dma_start(dbgRZ, rz[:])
            rz16 = seq.tile([1, R], BF16, tag=f"rz16{ri}")
            nc.vector.tensor_scalar(rz16[:], rz[:], 1.0 / 3.0, None, MULT)
            rzb_ps = psR.tile([H0, R], F32, tag="rzb")
            nc.tensor.matmul(rzb_ps[:], onec[:, 0:H0], rz16[:],
                             start=True, stop=True)
            rzb = seq.tile([H0, R], F32, tag=f"rzb_sb{ri}")
            nc.scalar.activation(rzb[:], rzb_ps[:], CPY)
            m = seq.tile([H0, R], F32, tag=f"m{ri}")
            nc.vector.tensor_mul(m[:], rzb[:], ht[ri][0:H0, :])
            if msum is None:
                msum = m
            else:
                m2 = seq.tile([H0, R], F32, tag=f"msum{ri}")
                nc.vector.tensor_add(m2[:], msum[:], m[:])
                msum = m2
        th = seq.tile([H0, R], BF16, tag="th")
        nc.scalar.activation(th[:], msum[:], TANH, scale=0.5)
        hpT = seq.tile([H0, R], BF16, tag="hpT")
        nc.vector.tensor_scalar(hpT[:], th[:], 0.5, 0.5, MULT, ADD)
        if DEBUG_DUMP:
            dHP = seq.tile([H0, R], F32, tag="dHP")
            nc.vector.tensor_scalar(dHP[:], hpT[:], 1.0, None, MULT)
            nc.sync.dma_start(dbgHP, dHP[:])
        ps_ctx.close()

        # ---- layer 1: project locally, transpose, AllGather, aggregate ------
        psB = ctx.enter_context(tc.tile_pool(name="psB", bufs=1, space="PSUM"))
        sup1p = psB.tile([H1, R], F32, tag="sup1p")
        nc.tensor.matmul(sup1p[:], wg0_sb[:], hpT[:], start=True, stop=True)
        sup1T = seq.tile([H1, R], BF16, tag="sup1T")
        nc.scalar.activation(sup1T[:], sup1p[:], CPY)
        sup1L = seq.tile([128, SLABC, H1], BF16, tag="sup1L")
        for q in range(SLABC):
            eng = nc.sync if q % 2 == 0 else nc.scalar
            eng.dma_start_transpose(sup1L[:, q, :],
                                    sup1T[:, q * 128:(q + 1) * 128])
        if DEBUG_DUMP:
            dS1L = seq.tile([128, SLABC, H1], F32, tag="dS1L")
            nc.vector.tensor_scalar(dS1L[:], sup1L[:], 1.0, None, MULT)
            nc.sync.dma_start(dbgS1L, dS1L[:])
        nc.sync.dma_start(cc1_in[:], sup1L[:])
        nc.gpsimd.collective_compute("AllGather", mybir.AluOpType.bypass,
                                     replica_groups=groups,
                                     ins=[cc1_in[:]], outs=[cc1_out[:]])
        for c in range(N_CORES):
            nc.sync.dma_start(sup1_all[:, c * SLABC:(c + 1) * SLABC, 0:H1],
                              cc1_out[c])

        if DEBUG_DUMP:
            dS1A = seq.tile([128, H1 + 1], F32, tag="dS1A")
            nc.vector.tensor_scalar(dS1A[:], sup1_all[:, 0, :], 1.0, None, MULT)
            nc.sync.dma_start(dbgS1A, dS1A[:])
        agg1 = psB.tile([H1 + 1, R], F32, tag="agg1")
        for jc in range(NJC):
            nc.tensor.matmul(agg1[:], sup1_all[:, jc, :],
                             adjres[:, jc // SLABC, jc % SLABC, :],
                             start=(jc == 0), stop=(jc == NJC - 1))
        degin = seq.tile([1, R], F32, tag="degin")
        nc.scalar.activation(degin[:], agg1[H1:H1 + 1, :], CPY)
        dinv = seq.tile([1, R], F32, tag="dinv")
        nc.vector.reciprocal_approx_fast(dinv[:], degin[:])
        dinv16 = seq.tile([1, R], BF16, tag="dinv16")
        nc.vector.tensor_scalar(dinv16[:], dinv[:], 1.0, None, MULT)
        dinvb_ps = psB.tile([H1, R], F32, tag="dinvb")
        nc.tensor.matmul(dinvb_ps[:], onec[:, 0:H1], dinv16[:],
                         start=True, stop=True)
        dinvb = seq.tile([H1, R], F32, tag="dinvb_sb")
        nc.scalar.activation(dinvb[:], dinvb_ps[:], CPY)
        m1 = seq.tile([H1, R], F32, tag="l1m")
        nc.vector.tensor_mul(m1[:], dinvb[:], agg1[0:H1, :])
        h1pT = seq.tile([H1, R], BF16, tag="h1pT")
        nc.scalar.activation(h1pT[:], m1[:], PRELU, bias=bg0_sb[:], scale=1.0,
                             alpha=SLOPE)
        if DEBUG_DUMP:
            dAGG = seq.tile([H1 + 1, R], F32, tag="dAGG")
            nc.scalar.activation(dAGG[:], agg1[:], CPY)
            nc.sync.dma_start(dbgAGG, dAGG[:])
            dH1P = seq.tile([H1, R], F32, tag="dH1P")
            nc.vector.tensor_scalar(dH1P[:], h1pT[:], 1.0, None, MULT)
            nc.sync.dma_start(dbgH1P, dH1P[:])

        # ---- layer 2 + residual --------------------------------------------
        sup2p = psB.tile([H2, R], F32, tag="sup2p")
        nc.tensor.matmul(sup2p[:], wg1_sb[:], h1pT[:], start=True, stop=True)
        sup2T = seq.tile([H2, R], BF16, tag="sup2T")
        nc.scalar.activation(sup2T[:], sup2p[:], CPY)
        sup2L = seq.tile([128, SLABC, H2], BF16, tag="sup2L")
        for q in range(SLABC):
            eng = nc.sync if q % 2 == 0 else nc.scalar
            eng.dma_start_transpose(sup2L[:, q, :],
                                    sup2T[:, q * 128:(q + 1) * 128])
        nc.sync.dma_start(cc2_in[:], sup2L[:])
        nc.gpsimd.collective_compute("AllGather", mybir.AluOpType.bypass,
                                     replica_groups=groups,
                                     ins=[cc2_in[:]], outs=[cc2_out[:]])
        for c in range(N_CORES):
            nc.sync.dma_start(sup2_all[:, c * SLABC:(c + 1) * SLABC, :],
                              cc2_out[c])

        if DEBUG_DUMP:
            dS2A = seq.tile([128, H2], F32, tag="dS2A")
            nc.vector.tensor_scalar(dS2A[:], sup2_all[:, 0, :], 1.0, None, MULT)
            nc.sync.dma_start(dbgS2A, dS2A[:])
        agg2 = psB.tile([H2, R], F32, tag="agg2")
        for jc in range(NJC):
            nc.tensor.matmul(agg2[:], sup2_all[:, jc, :],
                             adjres[:, jc // SLABC, jc % SLABC, :],
                             start=(jc == 0), stop=(jc == NJC - 1))
        resT = psB.tile([H2, R], F32, tag="resT")
        nc.tensor.matmul(resT[:], wrt_sb[:], h1pT[:], start=True, stop=True)

        m2t = seq.tile([H2, R], F32, tag="l2m")
        nc.vector.tensor_mul(m2t[:], dinvb[0:H2, :], agg2[:])
        t2 = seq.tile([H2, R], F32, tag="t2")
        nc.scalar.activation(t2[:], m2t[:], PRELU, bias=bg1_sb[:], scale=1.0,
                             alpha=SLOPE)
        if DEBUG_DUMP:
            dAGG2 = seq.tile([H2, R], F32, tag="dAGG2")
            nc.scalar.activation(dAGG2[:], agg2[:], CPY)
            nc.sync.dma_start(dbgAGG2, dAGG2[:])
            dRES = seq.tile([H2, R], F32, tag="dRES")
            nc.scalar.activation(dRES[:], resT[:], CPY)
            nc.sync.dma_start(dbgRES, dRES[:])
            nc.sync.dma_start(dbgT2, t2[:])
        fin = seq.tile([H2, R], F32, tag="fin")
        nc.vector.tensor_add(fin[:], t2[:], resT[:])
        fin2 = seq.tile([H2, R], F32, tag="fin2")
        nc.vector.tensor_scalar(fin2[:], fin[:], brc_sb[:], None, ADD)
        nc.sync.dma_start(outT[:], fin2[:])

    nc.compile()
    _model_cache["nc"] = nc
    return nc


def kernel(x, adj, W1, a1, W2, a2, W3, a3, Wg0, bg0, Wg1, bg1, Wr, br,
           relation):
    x = np.asarray(x, dtype=np.float32)
    adj = np.asarray(adj, dtype=np.float32)
    rel = int(np.asarray(relation))
    rel_list = [rel] + [r for r in range(3) if r != rel]
    Ws = [np.asarray(W, np.float32) for W in (W1, W2, W3)]
    As = [np.asarray(a, np.float32) for a in (a1, a2, a3)]

    # host prep: projections and score vectors (small)
    wh = [x @ Ws[r] for r in range(3)]                      # [N, 64] each
    s_src = [wh[r] @ As[r][:H0, 0] for r in range(3)]       # [N]
    s_dst = [wh[r] @ As[r][H0:, 0] for r in range(3)]       # [N]

    whcat = np.zeros((N, 200), np.float32)
    for ri, r in enumerate(rel_list):
        whcat[:, ri * 65:ri * 65 + 64] = wh[r]
        whcat[:, ri * 65 + 64] = 1.0
    # [4, 128, 8, 200]: row j = s4*1024 + k*128 + p
    whcat_sw = np.ascontiguousarray(
        whcat.astype(NPBF).reshape(4, 8, 128, 200).transpose(0, 2, 1, 3))

    # per-(relation, chunk) per-partition scalars [128, 96]
    sdst_c = np.concatenate(
        [s_dst[r].reshape(NJC, 128).T for r in rel_list], axis=1)
    sdst_c = np.ascontiguousarray(sdst_c.astype(np.float32))
    expd_c = np.exp(sdst_c).astype(np.float32)
    expd001_c = np.exp(0.01 * sdst_c).astype(np.float32)

    adj_bf = adj.astype(NPBF)
    in_maps = []
    for c in range(N_CORES):
        rows = slice(c * R, (c + 1) * R)
        # adjT[j, i] with j = s*512 + k*128 + p -> [3, 8, 128, 4, 512]
        adjt_c = adj_bf[rel_list][:, rows, :].transpose(0, 2, 1)  # [3,4096,512]
        adjt_c = np.ascontiguousarray(
            adjt_c.reshape(3, NSLAB, SLABC, 128, R).transpose(0, 1, 3, 2, 4))
        ssrcb_c = np.ascontiguousarray(np.broadcast_to(
            np.stack([s_src[r][rows] for r in rel_list])[:, None, :],
            (3, 128, R))).astype(np.float32)
        in_maps.append({
            "adjt": adjt_c,
            "whcat": whcat_sw,
            "ssrcb": ssrcb_c,
            "sdst": sdst_c,
            "expd": expd_c,
            "expd001": expd001_c,
            "wg0": np.asarray(Wg0, np.float32).astype(NPBF),
            "wg1": np.asarray(Wg1, np.float32).astype(NPBF),
            "wrt": np.ascontiguousarray(
                np.asarray(Wr, np.float32).T).astype(NPBF),
            "bg0": np.asarray(bg0, np.float32).reshape(H1, 1),
            "bg1": np.asarray(bg1, np.float32).reshape(H2, 1),
            "brc": np.asarray(br, np.float32).reshape(H2, 1),
        })

    nc = _build_model()
    kw = {}
    if os.environ.get("HRAN_TRACE"):
        _install_hook()
        kw = dict(trace=True, tmpdir=os.environ.get("HRAN_TRACE_DIR") or None)
    res = run_bass_kernel_spmd(nc, in_maps, core_ids=list(range(N_CORES)), **kw)
    if os.environ.get("HRAN_TRACE"):
        print(f"HW exec time: {res.exec_time_ns} ns")
    out = np.concatenate(
        [np.asarray(res.results[c]["outT"], np.float32).T
         for c in range(N_CORES)],
        axis=0)
    return out


def _install_hook():
    import antenv
    if "antenv.axon_hooks" in sys.modules:
        return
    from trn_agent_boot.trn_boot import _ntff_profile_via_ctypes
    hook = _ntff_profile_via_ctypes("/opt/axon/libaxon_pjrt.so")
    mod = types.ModuleType("antenv.axon_hooks")
    mod.get_axon_ntff_profile_hook = lambda: hook
    mod.set_axon_ntff_profile_hook = lambda h: None
    sys.modules["antenv.axon_hooks"] = mod
    antenv.axon_hooks = mod


# revision 20
# speedup vs baseline: 1.2934x; 1.0806x over previous
"""HRAN-GNN Trainium2 kernel: 8-core SPMD, row-sharded attention + GNN.

Per core c (rows i = [512c, 512c+512)), transposed layout [j-part, i-free].

Attention P = mask * exp(leaky(s_src_i + s_dst_j)), exact, via two paths
split across engines per 4-chunk slab [128, 4, 512]:
  - ACT path: lr_k = Prelu(ssrc + bias sdst_j); E = Exp(lr slab) slab-fused;
    P = E * adjT on DVE.  Prelu/Exp/Tanh/Copy share ONE act table set ->
    zero table swaps; sigmoid is computed as 0.5 + 0.5*tanh(x/2).
  - DVE path: exp(leaky(v)) = max(exp(v), exp(0.01 v)); both factor rank-1:
    E1 = exp(s_i)*exp(s_j) via tensor_scalar (bf16 4x mode), E2 likewise
    with 0.01-scaled scores; M = max(E1,E2); P = M * adjT.
PE contracts P chunks against stationary [wh_r | ones] -> ht[65, 512] PSUM;
the ones column gives softmax Z; 1/Z via reciprocal_approx_fast (SBUF only
-- it reads garbage from PSUM).

A dummy AllGather at kernel start absorbs the ~11 us first-collective launch
cost and inter-core skew while attention runs.

GNN layers: 1/deg comes precomputed from the host; its [64, R] broadcast is
built during attention.  Sender-side projection (sup1T = Wg0^T h'^T, one
matmul), bf16 cast, 4 XBAR DMA transposes (split across the SP/Act HWDGE
queues) -> [128, 4, 64] shard, one AllGather per layer, receivers use the
gathered tiles directly as aggregation stationaries against the resident
adjT of `relation`.  Layer-2 gather payload is [128, 4, 32].
"""
import os
import sys
import types

sys.path.insert(0, "/opt/trn_rl_repo")
sys.path.insert(0, "/root/.axon_site")

from contextlib import ExitStack
import numpy as np
import ml_dtypes

import concourse.bass as bass
import concourse.tile as tile
from concourse import bacc, mybir
from concourse.bass_utils import run_bass_kernel_spmd

F32 = mybir.dt.float32
BF16 = mybir.dt.bfloat16
NPBF = ml_dtypes.bfloat16

N = 4096
IN_F = 256
H0, H1, H2 = 64, 64, 32
SLOPE = 0.01
N_CORES = 8
R = N // N_CORES          # 512 rows per core
NJC = N // 128            # 32 j-chunks
NSLAB = 8                 # 8 slabs of 4 chunks per relation
SLABC = 4                 # chunks per slab

# per-(relation, slab) path: True = ACT path, False = DVE path
PATH_ACT = [
    [True, True, False, True, False, True, False, True],
    [True, True, False, True, False, True, False, True],
    [True, False, False, True, False, True, False, True],
]
# ACT-path slabs whose mask-multiply runs on GpSimd (Pool) instead of DVE
POOL_MASK = set()

_model_cache = {}
DEBUG_DUMP = False


def _build_model():
    if "nc" in _model_cache:
        return _model_cache["nc"]
    nc = bacc.Bacc("TRN2", target_bir_lowering=False, debug=False,
                   num_devices=N_CORES)

    adjt = nc.dram_tensor("adjt", [3, NSLAB, 128, SLABC, R], BF16,
                          kind="ExternalInput").ap()
    whcat = nc.dram_tensor("whcat", [4, 128, 8, 200], BF16,
                           kind="ExternalInput").ap()
    ssrcb = nc.dram_tensor("ssrcb", [3, 128, R], F32, kind="ExternalInput").ap()
    sdst = nc.dram_tensor("sdst", [128, 96], F32, kind="ExternalInput").ap()
    expd = nc.dram_tensor("expd", [128, 96], F32, kind="ExternalInput").ap()
    expd001 = nc.dram_tensor("expd001", [128, 96], F32,
                             kind="ExternalInput").ap()
    wg0 = nc.dram_tensor("wg0", [H1, H1], BF16, kind="ExternalInput").ap()
    wg1 = nc.dram_tensor("wg1", [H1, H2], BF16, kind="ExternalInput").ap()
    wrt = nc.dram_tensor("wrt", [H1, H2], BF16, kind="ExternalInput").ap()
    bg0 = nc.dram_tensor("bg0", [H1, 1], F32, kind="ExternalInput").ap()
    bg1 = nc.dram_tensor("bg1", [H2, 1], F32, kind="ExternalInput").ap()
    brc = nc.dram_tensor("brc", [H2, 1], F32, kind="ExternalInput").ap()
    dinvh = nc.dram_tensor("dinvh", [1, R], BF16, kind="ExternalInput").ap()
    outT = nc.dram_tensor("outT", [H2, R], F32, kind="ExternalOutput").ap()
    if DEBUG_DUMP:
        dbgP0 = nc.dram_tensor("dbgP0", [128, R], F32, kind="ExternalOutput").ap()
        dbgP2 = nc.dram_tensor("dbgP2", [128, R], F32, kind="ExternalOutput").ap()
        dbgZ = nc.dram_tensor("dbgZ", [1, R], F32, kind="ExternalOutput").ap()
        dbgRZ = nc.dram_tensor("dbgRZ", [1, R], F32, kind="ExternalOutput").ap()
        dbgHP = nc.dram_tensor("dbgHP", [H0, R], F32, kind="ExternalOutput").ap()
        dbgS1L = nc.dram_tensor("dbgS1L", [128, SLABC, H1], F32, kind="ExternalOutput").ap()
        dbgS1A = nc.dram_tensor("dbgS1A", [128, H1 + 1], F32, kind="ExternalOutput").ap()
        dbgAGG = nc.dram_tensor("dbgAGG", [H1 + 1, R], F32, kind="ExternalOutput").ap()
        dbgH1P = nc.dram_tensor("dbgH1P", [H1, R], F32, kind="ExternalOutput").ap()
        dbgS2A = nc.dram_tensor("dbgS2A", [128, H2], F32, kind="ExternalOutput").ap()
        dbgAGG2 = nc.dram_tensor("dbgAGG2", [H2, R], F32, kind="ExternalOutput").ap()
        dbgRES = nc.dram_tensor("dbgRES", [H2, R], F32, kind="ExternalOutput").ap()
        dbgT2 = nc.dram_tensor("dbgT2", [H2, R], F32, kind="ExternalOutput").ap()

    cc1_in = nc.dram_tensor("cc1_in", [128, SLABC, H1], BF16).ap()
    cc1_out = nc.dram_tensor("cc1_out", [N_CORES, 128, SLABC, H1], BF16,
                             addr_space="Shared").ap()
    ccw_in = nc.dram_tensor("ccw_in", [1, 16], BF16).ap()
    ccw_out = nc.dram_tensor("ccw_out", [N_CORES, 1, 16], BF16,
                             addr_space="Shared").ap()
    cc2_in = nc.dram_tensor("cc2_in", [128, SLABC, H2], BF16).ap()
    cc2_out = nc.dram_tensor("cc2_out", [N_CORES, 128, SLABC, H2], BF16,
                             addr_space="Shared").ap()
    groups = [list(range(N_CORES))]

    PRELU = mybir.ActivationFunctionType.Prelu
    EXP = mybir.ActivationFunctionType.Exp
    TANH = mybir.ActivationFunctionType.Tanh
    CPY = mybir.ActivationFunctionType.Copy
    MULT = mybir.AluOpType.mult
    ADD = mybir.AluOpType.add

    with tile.TileContext(nc) as tc, ExitStack() as ctx:
        resid = ctx.enter_context(tc.tile_pool(name="resid", bufs=1))
        small = ctx.enter_context(tc.tile_pool(name="small", bufs=1))
        stream = ctx.enter_context(tc.tile_pool(name="stream", bufs=4))
        lrp = ctx.enter_context(tc.tile_pool(name="lrp", bufs=3))
        ep = ctx.enter_context(tc.tile_pool(name="ep", bufs=3))
        e12p = ctx.enter_context(tc.tile_pool(name="e12p", bufs=2))
        mp = ctx.enter_context(tc.tile_pool(name="mp", bufs=2))
        pp = ctx.enter_context(tc.tile_pool(name="pp", bufs=3))
        seq = ctx.enter_context(tc.tile_pool(name="seq", bufs=1))

        # warm up the collective rings early (first CC op pays ~11us launch
        # + slow transfer; do it on garbage while attention runs)
        nc.gpsimd.collective_compute("AllGather", mybir.AluOpType.bypass,
                                     replica_groups=groups,
                                     ins=[ccw_in[:]], outs=[ccw_out[:]])

        # ---- resident loads -------------------------------------------------
        adjres = resid.tile([128, NSLAB, SLABC, R], BF16)     # relation's adjT
        nc.sync.dma_start(adjres[:, 0], adjt[0, 0])
        wh_sb = resid.tile([128, 32, 200], BF16)
        for s4 in range(4):
            nc.sync.dma_start(wh_sb[:, s4 * 8:(s4 + 1) * 8, :], whcat[s4])
        ssrc_sb = [resid.tile([128, R], F32, tag=f"ssrc{ri}", name=f"ssrc{ri}")
                   for ri in range(3)]
        for ri in range(3):
            nc.sync.dma_start(ssrc_sb[ri][:], ssrcb[ri])
        sdst_sb = resid.tile([128, 96], F32)
        nc.sync.dma_start(sdst_sb[:], sdst[:])
        expd_sb = resid.tile([128, 96], F32)
        nc.sync.dma_start(expd_sb[:], expd[:])
        expd001_sb = resid.tile([128, 96], F32)
        nc.sync.dma_start(expd001_sb[:], expd001[:])
        for s in range(1, NSLAB):
            nc.sync.dma_start(adjres[:, s], adjt[0, s])

        wg0_sb = small.tile([H1, H1], BF16, tag="wg0")
        nc.sync.dma_start(wg0_sb[:], wg0[:])
        wg1_sb = small.tile([H1, H2], BF16, tag="wg1")
        nc.sync.dma_start(wg1_sb[:], wg1[:])
        wrt_sb = small.tile([H1, H2], BF16, tag="wrt")
        nc.sync.dma_start(wrt_sb[:], wrt[:])
        bg0_sb = small.tile([H1, 1], F32, tag="bg0")
        nc.sync.dma_start(bg0_sb[:], bg0[:])
        bg1_sb = small.tile([H2, 1], F32, tag="bg1")
        nc.sync.dma_start(bg1_sb[:], bg1[:])
        brc_sb = small.tile([H2, 1], F32, tag="brc")
        nc.sync.dma_start(brc_sb[:], brc[:])
        onec = small.tile([1, H1], BF16, tag="onec")
        nc.vector.memset(onec[:], 1.0)

        # degree normalization broadcast, built early from host 1/deg
        psD = ctx.enter_context(tc.tile_pool(name="psD", bufs=1, space="PSUM"))
        dinv16 = seq.tile([1, R], BF16, tag="dinv16")
        nc.sync.dma_start(dinv16[:], dinvh[:])
        dinvb_ps = psD.tile([H1, R], F32, tag="dinvb")
        nc.tensor.matmul(dinvb_ps[:], onec[:, 0:H1], dinv16[:],
                         start=True, stop=True)
        dinvb = seq.tile([H1, R], F32, tag="dinvb_sb")
        nc.scalar.activation(dinvb[:], dinvb_ps[:], CPY)

        # rank-1 factors for the DVE path, built on device:
        #   A_b = exp(s_src) broadcast, a_b = exp(0.01 s_src) broadcast
        A_b = [resid.tile([128, R], BF16, tag=f"Ab{ri}", name=f"Ab{ri}")
               for ri in range(3)]
        a_b = [resid.tile([128, R], BF16, tag=f"ab{ri}", name=f"ab{ri}")
               for ri in range(3)]
        for ri in range(3):
            nc.scalar.activation(A_b[ri][:], ssrc_sb[ri][:], EXP)
            nc.scalar.activation(a_b[ri][:], ssrc_sb[ri][:], EXP, scale=0.01)

        # gathered aggregation stationaries
        sup1_all = resid.tile([128, NJC, H1], BF16)
        sup2_all = resid.tile([128, NJC, H2], BF16)

        # ---- phase A: masked-softmax attention, all 3 relations -------------
        ps_ctx = ExitStack()
        psA = ps_ctx.enter_context(tc.tile_pool(name="psA", bufs=1,
                                                space="PSUM"))
        psR = ps_ctx.enter_context(tc.tile_pool(name="psR", bufs=2,
                                                space="PSUM"))
        ht = [psA.tile([H0 + 1, R], F32, tag=f"ht{ri}", name=f"ht{ri}")
              for ri in range(3)]
        for ri in range(3):
            for s in range(NSLAB):
                if ri == 0:
                    at = adjres[:, s]
                else:
                    att = stream.tile([128, SLABC, R], BF16, tag="adj_stream",
                                      name=f"adj_{ri}_{s}")
                    nc.sync.dma_start(att[:], adjt[ri, s])
                    at = att[:]
                p = pp.tile([128, SLABC, R], BF16, tag="p", name=f"p_{ri}_{s}")
                if PATH_ACT[ri][s]:
                    lr = lrp.tile([128, SLABC, R], F32, tag="lr",
                                  name=f"lr_{ri}_{s}")
                    for k in range(SLABC):
                        col = ri * 32 + s * SLABC + k
                        nc.scalar.activation(
                            lr[:, k, :], ssrc_sb[ri][:], PRELU,
                            bias=sdst_sb[:, col:col + 1], scale=1.0,
                            alpha=SLOPE)
                    ex = ep.tile([128, SLABC, R], BF16, tag="ex",
                                 name=f"ex_{ri}_{s}")
                    nc.scalar.activation(ex[:], lr[:], EXP)
                    if (ri, s) in POOL_MASK:
                        nc.gpsimd.tensor_mul(p[:], ex[:], at)
                    else:
                        nc.vector.tensor_mul(p[:], ex[:], at)
                else:
                    e1 = e12p.tile([128, SLABC, R], BF16, tag="e1",
                                   name=f"e1_{ri}_{s}")
                    e2 = e12p.tile([128, SLABC, R], BF16, tag="e2",
                                   name=f"e2_{ri}_{s}")
                    for k in range(SLABC):
                        col = ri * 32 + s * SLABC + k
                        nc.vector.tensor_scalar(
                            e1[:, k, :], A_b[ri][:],
                            expd_sb[:, col:col + 1], None, MULT)
                        nc.vector.tensor_scalar(
                            e2[:, k, :], a_b[ri][:],
                            expd001_sb[:, col:col + 1], None, MULT)
                    m = mp.tile([128, SLABC, R], BF16, tag="m",
                                name=f"m_{ri}_{s}")
                    nc.vector.tensor_max(m[:], e1[:], e2[:])
                    nc.vector.tensor_mul(p[:], m[:], at)
                if DEBUG_DUMP and ri == 0 and s in (0, 2):
                    dP = seq.tile([128, R], F32, tag=f"dP{s}")
                    nc.vector.tensor_scalar(dP[:], p[:, 0, :], 1.0, None, MULT)
                    nc.sync.dma_start(dbgP0 if s == 0 else dbgP2, dP[:])
                for k in range(SLABC):
                    jc = s * SLABC + k
                    nc.tensor.matmul(ht[ri][:],
                                     wh_sb[:, jc, ri * 65:ri * 65 + 65],
                                     p[:, k, :],
                                     start=(jc == 0), stop=(jc == NJC - 1))

        # combine: h' = sigmoid(mean of normalized heads) = 0.5+0.5tanh(x/2)
        msum = None
        for ri in range(3):
            zin = seq.tile([1, R], F32, tag=f"zin{ri}")
            nc.scalar.activation(zin[:], ht[ri][H0:H0 + 1, :], CPY)
            rz = seq.tile([1, R], F32, tag=f"rz{ri}")
            nc.vector.reciprocal_approx_fast(rz[:], zin[:])
            if DEBUG_DUMP and ri == 0:
                dZ = seq.tile([1, R], F32, tag="dZ")
                nc.scalar.activation(dZ[:], ht[ri][H0:H0 + 1, :], CPY)
                nc.sync.dma_start(dbgZ, dZ[:])
                nc.sync.dma_start(dbgRZ, rz[:])
            rz16 = seq.tile([1, R], BF16, tag=f"rz16{ri}")
            nc.vector.tensor_scalar(rz16[:], rz[:], 1.0 / 3.0, None, MULT)
            rzb_ps = psR.tile([H0, R], F32, tag="rzb")
            nc.tensor.matmul(rzb_ps[:], onec[:, 0:H0], rz16[:],
                             start=True, stop=True)
            rzb = seq.tile([H0, R], F32, tag=f"rzb_sb{ri}")
            nc.scalar.activation(rzb[:], rzb_ps[:], CPY)
            m = seq.tile([H0, R], F32, tag=f"m{ri}")
            nc.vector.tensor_mul(m[:], rzb[:], ht[ri][0:H0, :])
            if msum is None:
                msum = m
            else:
                m2 = seq.tile([H0, R], F32, tag=f"msum{ri}")
                nc.vector.tensor_add(m2[:], msum[:], m[:])
                msum = m2
        th = seq.tile([H0, R], BF16, tag="th")
        nc.scalar.activation(th[:], msum[:], TANH, scale=0.5)
        hpT = seq.tile([H0, R], BF16, tag="hpT")
        nc.vector.tensor_scalar(hpT[:], th[:], 0.5, 0.5, MULT, ADD)
        if DEBUG_DUMP:
            dHP = seq.tile([H0, R], F32, tag="dHP")
            nc.vector.tensor_scalar(dHP[:], hpT[:], 1.0, None, MULT)
            nc.sync.dma_start(dbgHP, dHP[:])
        ps_ctx.close()

        # ---- layer 1: project locally, transpose, AllGather, aggregate ------
        psB = ctx.enter_context(tc.tile_pool(name="psB", bufs=1, space="PSUM"))
        sup1p = psB.tile([H1, R], F32, tag="sup1p")
        nc.tensor.matmul(sup1p[:], wg0_sb[:], hpT[:], start=True, stop=True)
        sup1T = seq.tile([H1, R], BF16, tag="sup1T")
        nc.scalar.activation(sup1T[:], sup1p[:], CPY)
        sup1L = seq.tile([128, SLABC, H1], BF16, tag="sup1L")
        nc.sync.dma_start_transpose(sup1L[:, :, :], sup1T[:])
        if DEBUG_DUMP:
            dS1L = seq.tile([128, SLABC, H1], F32, tag="dS1L")
            nc.vector.tensor_scalar(dS1L[:], sup1L[:], 1.0, None, MULT)
            nc.sync.dma_start(dbgS1L, dS1L[:])
        nc.sync.dma_start(cc1_in[:], sup1L[:])
        nc.gpsimd.collective_compute("AllGather", mybir.AluOpType.bypass,
                                     replica_groups=groups,
                                     ins=[cc1_in[:]], outs=[cc1_out[:]])
        for c in range(N_CORES):
            nc.sync.dma_start(sup1_all[:, c * SLABC:(c + 1) * SLABC, 0:H1],
                              cc1_out[c])

        if DEBUG_DUMP:
            dS1A = seq.tile([128, H1 + 1], F32, tag="dS1A")
            nc.vector.tensor_scalar(dS1A[:], sup1_all[:, 0, :], 1.0, None, MULT)
            nc.sync.dma_start(dbgS1A, dS1A[:])
        agg1 = psB.tile([H1 + 1, R], F32, tag="agg1")
        for jc in range(NJC):
            nc.tensor.matmul(agg1[:], sup1_all[:, jc, :],
                             adjres[:, jc // SLABC, jc % SLABC, :],
                             start=(jc == 0), stop=(jc == NJC - 1))
        degin = seq.tile([1, R], F32, tag="degin")
        nc.scalar.activation(degin[:], agg1[H1:H1 + 1, :], CPY)
        dinv = seq.tile([1, R], F32, tag="dinv")
        nc.vector.reciprocal_approx_fast(dinv[:], degin[:])
        dinv16 = seq.tile([1, R], BF16, tag="dinv16")
        nc.vector.tensor_scalar(dinv16[:], dinv[:], 1.0, None, MULT)
        dinvb_ps = psB.tile([H1, R], F32, tag="dinvb")
        nc.tensor.matmul(dinvb_ps[:], onec[:, 0:H1], dinv16[:],
                         start=True, stop=True)
        dinvb = seq.tile([H1, R], F32, tag="dinvb_sb")
        nc.scalar.activation(dinvb[:], dinvb_ps[:], CPY)
        m1 = seq.tile([H1, R], F32, tag="l1m")
        nc.vector.tensor_mul(m1[:], dinvb[:], agg1[0:H1, :])
        h1pT = seq.tile([H1, R], BF16, tag="h1pT")
        nc.scalar.activation(h1pT[:], m1[:], PRELU, bias=bg0_sb[:], scale=1.0,
                             alpha=SLOPE)
        if DEBUG_DUMP:
            dAGG = seq.tile([H1 + 1, R], F32, tag="dAGG")
            nc.scalar.activation(dAGG[:], agg1[:], CPY)
            nc.sync.dma_start(dbgAGG, dAGG[:])
            dH1P = seq.tile([H1, R], F32, tag="dH1P")
            nc.vector.tensor_scalar(dH1P[:], h1pT[:], 1.0, None, MULT)
            nc.sync.dma_start(dbgH1P, dH1P[:])

        # ---- layer 2 + residual --------------------------------------------
        sup2p = psB.tile([H2, R], F32, tag="sup2p")
        nc.tensor.matmul(sup2p[:], wg1_sb[:], h1pT[:], start=True, stop=True)
        sup2T = seq.tile([H2, R], BF16, tag="sup2T")
        nc.scalar.activation(sup2T[:], sup2p[:], CPY)
        sup2L = seq.tile([128, SLABC, H2], BF16, tag="sup2L")
        nc.sync.dma_start_transpose(sup2L[:, :, :], sup2T[:])
        nc.sync.dma_start(cc2_in[:], sup2L[:])
        nc.gpsimd.collective_compute("AllGather", mybir.AluOpType.bypass,
                                     replica_groups=groups,
                                     ins=[cc2_in[:]], outs=[cc2_out[:]])
        for c in range(N_CORES):
            nc.sync.dma_start(sup2_all[:, c * SLABC:(c + 1) * SLABC, :],
                              cc2_out[c])

        if DEBUG_DUMP:
            dS2A = seq.tile([128, H2], F32, tag="dS2A")
            nc.vector.tensor_scalar(dS2A[:], sup2_all[:, 0, :], 1.0, None, MULT)
            nc.sync.dma_start(dbgS2A, dS2A[:])
        agg2 = psB.tile([H2, R], F32, tag="agg2")
        for jc in range(NJC):
            nc.tensor.matmul(agg2[:], sup2_all[:, jc, :],
                             adjres[:, jc // SLABC, jc % SLABC, :],
                             start=(jc == 0), stop=(jc == NJC - 1))
        resT = psB.tile([H2, R], F32, tag="resT")
        nc.tensor.matmul(resT[:], wrt_sb[:], h1pT[:], start=True, stop=True)

        m2t = seq.tile([H2, R], F32, tag="l2m")
        nc.vector.tensor_mul(m2t[:], dinvb[0:H2, :], agg2[:])
        t2 = seq.tile([H2, R], F32, tag="t2")
        nc.scalar.activation(t2[:], m2t[:], PRELU, bias=bg1_sb[:], scale=1.0,
                             alpha=SLOPE)
        if DEBUG_DUMP:
            dAGG2 = seq.tile([H2, R], F32, tag="dAGG2")
            nc.scalar.activation(dAGG2[:], agg2[:], CPY)
            nc.sync.dma_start(dbgAGG2, dAGG2[:])
            dRES = seq.tile([H2, R], F32, tag="dRES")
            nc.scalar.activation(dRES[:], resT[:], CPY)
            nc.sync.dma_start(dbgRES, dRES[:])
            nc.sync.dma_start(dbgT2, t2[:])
        fin = seq.tile([H2, R], F32, tag="fin")
        nc.vector.tensor_add(fin[:], t2[:], resT[:])
        fin2 = seq.tile([H2, R], F32, tag="fin2")
        nc.vector.tensor_scalar(fin2[:], fin[:], brc_sb[:], None, ADD)
        nc.sync.dma_start(outT[:], fin2[:])

    nc.compile()
    _model_cache["nc"] = nc
    return nc


def kernel(x, adj, W1, a1, W2, a2, W3, a3, Wg0, bg0, Wg1, bg1, Wr, br,
           relation):
    x = np.asarray(x, dtype=np.float32)
    adj = np.asarray(adj, dtype=np.float32)
    rel = int(np.asarray(relation))
    rel_list = [rel] + [r for r in range(3) if r != rel]
    Ws = [np.asarray(W, np.float32) for W in (W1, W2, W3)]
    As = [np.asarray(a, np.float32) for a in (a1, a2, a3)]

    # host prep: projections and score vectors (small)
    wh = [x @ Ws[r] for r in range(3)]                      # [N, 64] each
    s_src = [wh[r] @ As[r][:H0, 0] for r in range(3)]       # [N]
    s_dst = [wh[r] @ As[r][H0:, 0] for r in range(3)]       # [N]

    whcat = np.zeros((N, 200), np.float32)
    for ri, r in enumerate(rel_list):
        whcat[:, ri * 65:ri * 65 + 64] = wh[r]
        whcat[:, ri * 65 + 64] = 1.0
    # [4, 128, 8, 200]: row j = s4*1024 + k*128 + p
    whcat_sw = np.ascontiguousarray(
        whcat.astype(NPBF).reshape(4, 8, 128, 200).transpose(0, 2, 1, 3))

    # per-(relation, chunk) per-partition scalars [128, 96]
    sdst_c = np.concatenate(
        [s_dst[r].reshape(NJC, 128).T for r in rel_list], axis=1)
    sdst_c = np.ascontiguousarray(sdst_c.astype(np.float32))
    expd_c = np.exp(sdst_c).astype(np.float32)
    expd001_c = np.exp(0.01 * sdst_c).astype(np.float32)

    adj_bf = adj.astype(NPBF)
    in_maps = []
    for c in range(N_CORES):
        rows = slice(c * R, (c + 1) * R)
        # adjT[j, i] with j = s*512 + k*128 + p -> [3, 8, 128, 4, 512]
        adjt_c = adj_bf[rel_list][:, rows, :].transpose(0, 2, 1)  # [3,4096,512]
        adjt_c = np.ascontiguousarray(
            adjt_c.reshape(3, NSLAB, SLABC, 128, R).transpose(0, 1, 3, 2, 4))
        ssrcb_c = np.ascontiguousarray(np.broadcast_to(
            np.stack([s_src[r][rows] for r in rel_list])[:, None, :],
            (3, 128, R))).astype(np.float32)
        deg_c = adj[rel][rows, :].sum(axis=1)
        in_maps.append({
            "adjt": adjt_c,
            "dinvh": np.where(deg_c > 0, 1.0 / np.maximum(deg_c, 1e-9),
                              0.0)[None, :].astype(NPBF),
            "whcat": whcat_sw,
            "ssrcb": ssrcb_c,
            "sdst": sdst_c,
            "expd": expd_c,
            "expd001": expd001_c,
            "wg0": np.asarray(Wg0, np.float32).astype(NPBF),
            "wg1": np.asarray(Wg1, np.float32).astype(NPBF),
            "wrt": np.ascontiguousarray(
                np.asarray(Wr, np.float32).T).astype(NPBF),
            "bg0": np.asarray(bg0, np.float32).reshape(H1, 1),
            "bg1": np.asarray(bg1, np.float32).reshape(H2, 1),
            "brc": np.asarray(br, np.float32).reshape(H2, 1),
        })

    nc = _build_model()
    kw = {}
    if os.environ.get("HRAN_TRACE"):
        _install_hook()
        kw = dict(trace=True, tmpdir=os.environ.get("HRAN_TRACE_DIR") or None)
    res = run_bass_kernel_spmd(nc, in_maps, core_ids=list(range(N_CORES)), **kw)
    if os.environ.get("HRAN_TRACE"):
        print(f"HW exec time: {res.exec_time_ns} ns")
    out = np.concatenate(
        [np.asarray(res.results[c]["outT"], np.float32).T
         for c in range(N_CORES)],
        axis=0)
    return out


def _install_hook():
    import antenv
    if "antenv.axon_hooks" in sys.modules:
        return
    from trn_agent_boot.trn_boot import _ntff_profile_via_ctypes
    hook = _ntff_profile_via_ctypes("/opt/axon/libaxon_pjrt.so")
    mod = types.ModuleType("antenv.axon_hooks")
    mod.get_axon_ntff_profile_hook = lambda: hook
    mod.set_axon_ntff_profile_hook = lambda h: None
    sys.modules["antenv.axon_hooks"] = mod
    antenv.axon_hooks = mod
